# revision 1
# baseline (speedup 1.0000x reference)
"""Trainium2 Bass kernel for a GCN layer:

    out = segment_sum(support[edge_col] * edge_val, edge_row) + bias
    where support = vertex @ W

Strategy (8 NeuronCores, SPMD):
  - Destination nodes are row-partitioned: core c owns rows
    [c*12500, (c+1)*12500).
  - Host pre-sorts edges by destination.  Each core's (padded) edge
    list is cut into chunks of 128 consecutive dest-sorted edges; a
    chunk may span adjacent 128-row destination windows.  Chunk counts
    are equalized across cores so one SPMD program serves all 8.
  - On device, per chunk of 128 edges:
      * indirect-DMA gather of the 128 source vertex rows (one index
        per partition — the only offset layout the SWDGE ucode honors)
      * for each destination window the chunk touches (usually 1,
        sometimes 2): build a val-weighted one-hot matrix
        oh[e, j] = (dloc[e] == j) * val[e] (val zeroed for edges of
        other windows) in one fused DVE tensor_scalar op, then
        matmul  aggT[feat, dest] += vertexG[e, feat].T @ oh[e, dest]
        accumulating in PSUM over all incidences of the window
  - Per window: out[dest, :] = (aggT.T @ W) + bias, DMA'd to DRAM.
    Aggregation happens in input-feature space BEFORE applying W, so
    no support table is materialized and no collective is needed.
"""

import numpy as np

N_NODES = 100000
N_EDGES = 1600000
IN_F = 128
OUT_F = 64
N_CORES = 8
P = 128
N_SHARD = N_NODES // N_CORES            # 12500 destination rows per core
N_WIN = (N_SHARD + P - 1) // P          # 98 destination windows per core


def _preprocess(edge_row, edge_col, edge_val, n_shard=N_SHARD, n_cores=N_CORES):
    """Sort edges by destination and build, per core:
      cols   [128, nch]  int32  — source node id of edge (p, chunk c)
      dloc   [128, ninc] f32    — window-local dest row (0..127)
      vals   [128, ninc] f32    — edge weight, 0 for padding / other window
      incidences: list of (chunk_idx, window_idx, start, stop) shared by
      all cores (program structure)."""
    n_win = (n_shard + P - 1) // P
    order = np.argsort(edge_row, kind="stable")
    rows_s = np.asarray(edge_row, dtype=np.int64)[order]
    cols_s = np.asarray(edge_col, dtype=np.int32)[order]
    vals_s = np.asarray(edge_val, dtype=np.float32)[order]

    core = rows_s // n_shard
    local = rows_s - core * n_shard

    counts_core = np.bincount(core, minlength=n_cores)
    nch = int(np.ceil(counts_core.max() / P))
    epc = nch * P                                  # padded edges per core

    # padded per-core arrays, edge i of core c at flat position c*epc+i
    core_starts = np.zeros(n_cores, np.int64)
    core_starts[1:] = np.cumsum(counts_core)[:-1]
    pos = np.arange(rows_s.size, dtype=np.int64) - core_starts[core]
    flat = core * epc + pos

    cols_p = np.zeros((n_cores, epc), np.int32)
    vals_p = np.zeros((n_cores, epc), np.float32)
    win_p = np.full((n_cores, epc), -1, np.int64)   # -1 = padding
    dloc_p = np.zeros((n_cores, epc), np.float32)
    cols_p.reshape(-1)[flat] = cols_s
    vals_p.reshape(-1)[flat] = vals_s
    win_p.reshape(-1)[flat] = local // P
    dloc_p.reshape(-1)[flat] = (local % P).astype(np.float32)

    # windows touched by each chunk, per core -> union across cores
    win_c = win_p.reshape(n_cores, nch, P)
    # incidence (c, w) if any core has an edge of window w in chunk c
    inc_mask = np.zeros((nch, n_win), bool)
    for w in range(n_win):
        inc_mask[:, w] = np.any(win_c == w, axis=(0, 2))
    # ensure every window has at least one incidence (empty windows)
    for w in range(n_win):
        if not inc_mask[:, w].any():
            inc_mask[min(w, nch - 1), w] = True

    incidences = []          # (chunk, window, start, stop)
    first_seen = {}
    last_seen = {}
    for c in range(nch):
        for w in np.nonzero(inc_mask[c])[0]:
            w = int(w)
            if w not in first_seen:
                first_seen[w] = len(incidences)
            last_seen[w] = len(incidences)
            incidences.append([c, w, False, False])
    for w, i in first_seen.items():
        incidences[i][2] = True
    for w, i in last_seen.items():
        incidences[i][3] = True
    ninc = len(incidences)

    # per-incidence dloc/val columns
    dloc_i = np.zeros((n_cores, ninc, P), np.float32)
    vals_i = np.zeros((n_cores, ninc, P), np.float32)
    for i, (c, w, _, _) in enumerate(incidences):
        m = win_c[:, c, :] == w                    # [n_cores, P]
        dloc_i[:, i, :] = np.where(m, win_c[:, c, :] * 0 + dloc_p.reshape(
            n_cores, nch, P)[:, c, :], 0.0)
        vals_i[:, i, :] = np.where(m, vals_p.reshape(
            n_cores, nch, P)[:, c, :], 0.0)

    cols_t = np.ascontiguousarray(
        cols_p.reshape(n_cores, nch, P).transpose(0, 2, 1))
    dloc_t = np.ascontiguousarray(dloc_i.transpose(0, 2, 1))
    vals_t = np.ascontiguousarray(vals_i.transpose(0, 2, 1))
    incs = [(int(c), int(w), bool(s0), bool(s1)) for c, w, s0, s1 in incidences]
    return cols_t, dloc_t, vals_t, incs, nch


def _build_nc(nch, incidences, in_f=IN_F, out_f=OUT_F, n_shard=N_SHARD,
              n_nodes=N_NODES, gather_bufs=24):
    import concourse.bacc as bacc
    import concourse.bass as bass
    import concourse.mybir as mybir
    import concourse.tile as tile

    f32 = mybir.dt.float32
    i32 = mybir.dt.int32
    nc = bacc.Bacc("TRN2", target_bir_lowering=False, debug=False,
                   enable_asserts=False)

    ninc = len(incidences)
    vertex = nc.dram_tensor("vertex", [n_nodes, in_f], f32,
                            kind="ExternalInput").ap()
    wmat = nc.dram_tensor("wmat", [in_f, out_f], f32, kind="ExternalInput").ap()
    bias_rep = nc.dram_tensor("bias_rep", [P, out_f], f32,
                              kind="ExternalInput").ap()
    iota = nc.dram_tensor("iota", [P, P], f32, kind="ExternalInput").ap()
    cols = nc.dram_tensor("cols", [P, nch], i32, kind="ExternalInput").ap()
    dloc = nc.dram_tensor("dloc", [P, ninc], f32, kind="ExternalInput").ap()
    vals = nc.dram_tensor("vals", [P, ninc], f32, kind="ExternalInput").ap()
    out = nc.dram_tensor("out", [n_shard, out_f], f32, kind="ExternalOutput").ap()

    # max simultaneously-open destination windows (PSUM tiles needed)
    open_w = set()
    agg_bufs = 2
    for rec in incidences:
        c, w, start, stop = rec[0], rec[1], rec[2], rec[3]
        if start:
            open_w.add(w)
            agg_bufs = max(agg_bufs, len(open_w))
        if stop:
            open_w.discard(w)
    agg_bufs = min(max(agg_bufs, 4), 5)

    with tile.TileContext(nc) as tc:
        with (
            tc.tile_pool(name="const", bufs=1) as cpool,
            tc.tile_pool(name="meta", bufs=1) as mpool,
            tc.tile_pool(name="gather", bufs=gather_bufs) as gpool,
            tc.tile_pool(name="oh", bufs=12) as opool,
            tc.tile_pool(name="evac", bufs=6) as epool,
            tc.tile_pool(name="agg_psum", bufs=agg_bufs, space="PSUM") as agg_pp,
            tc.tile_pool(name="out_psum", bufs=2, space="PSUM") as out_pp,
            tc.tile_pool(name="iota_psum", bufs=1, space="PSUM") as ipool,
        ):
            w_sb = cpool.tile([in_f, out_f], f32)
            nc.sync.dma_start(out=w_sb[:], in_=wmat[:])
            bias_sb = cpool.tile([P, out_f], f32)
            nc.sync.dma_start(out=bias_sb[:], in_=bias_rep[:])
            iota_sb = cpool.tile([P, P], f32)
            nc.sync.dma_start(out=iota_sb[:], in_=iota[:])
            iota_ps = ipool.tile([P, P], f32)
            nc.vector.tensor_copy(out=iota_ps[:], in_=iota_sb[:])
            cols_sb = mpool.tile([P, nch], i32)
            nc.sync.dma_start(out=cols_sb[:], in_=cols[:])
            dloc_sb = mpool.tile([P, ninc], f32)
            nc.sync.dma_start(out=dloc_sb[:], in_=dloc[:])
            vals_sb = mpool.tile([P, ninc], f32)
            nc.sync.dma_start(out=vals_sb[:], in_=vals[:])

            # one gather per chunk ([128,1] offsets — the only layout the
            # SWDGE indirect ucode honors)
            gtiles = []
            for c in range(nch):
                gbuf = gpool.tile([P, in_f], f32, tag="gbuf")
                nc.gpsimd.indirect_dma_start(
                    out=gbuf[:],
                    out_offset=None,
                    in_=vertex[:],
                    in_offset=bass.IndirectOffsetOnAxis(
                        ap=cols_sb[:, c:c + 1], axis=0),
                )
                gtiles.append(gbuf)

            agg_by_win = {}
            for i, (c, w, start, stop) in enumerate(incidences):
                if start:
                    aggT = agg_pp.tile([in_f, P], f32, tag="aggT")
                    agg_by_win[w] = aggT
                else:
                    aggT = agg_by_win[w]
                oh = opool.tile([P, P], f32, tag="oh")
                nc.vector.tensor_scalar(
                    out=oh[:],
                    in0=iota_ps[:],
                    scalar1=dloc_sb[:, i:i + 1],
                    scalar2=vals_sb[:, i:i + 1],
                    op0=mybir.AluOpType.is_equal,
                    op1=mybir.AluOpType.mult,
                )
                nc.tensor.matmul(
                    out=aggT[:],
                    lhsT=gtiles[c][:],
                    rhs=oh[:],
                    start=start,
                    stop=stop,
                )
                if stop:
                    aggT_sb = epool.tile([in_f, P], f32, tag="aggT_sb")
                    nc.vector.tensor_copy(out=aggT_sb[:], in_=aggT[:])
                    outw = out_pp.tile([P, out_f], f32, tag="outw")
                    nc.tensor.matmul(out=outw[:], lhsT=aggT_sb[:], rhs=w_sb[:],
                                     start=True, stop=True)
                    out_sb = epool.tile([P, out_f], f32, tag="out_sb")
                    nc.vector.tensor_tensor(out=out_sb[:], in0=outw[:],
                                            in1=bias_sb[:],
                                            op=mybir.AluOpType.add)
                    rows = min(P, n_shard - w * P)
                    nc.sync.dma_start(out=out[w * P: w * P + rows, :],
                                      in_=out_sb[:rows, :])

    nc.compile()
    return nc


def _make_in_maps(vertex, weights, bias, cols_t, dloc_t, vals_t,
                  n_cores=N_CORES):
    iota = np.tile(np.arange(P, dtype=np.float32)[None, :], (P, 1))
    bias_rep = np.ascontiguousarray(np.tile(np.asarray(bias, np.float32)[None, :],
                                            (P, 1)))
    vertex = np.ascontiguousarray(np.asarray(vertex, np.float32))
    weights = np.ascontiguousarray(np.asarray(weights, np.float32))
    return [
        {
            "vertex": vertex,
            "wmat": weights,
            "bias_rep": bias_rep,
            "iota": iota,
            "cols": cols_t[c],
            "dloc": dloc_t[c],
            "vals": vals_t[c],
        }
        for c in range(n_cores)
    ]


def _run(nc, in_maps, trace=False, tmpdir=None):
    from concourse import bass_utils
    from concourse.bass_interp import get_hw_module

    old_m = nc.m
    nc.m = get_hw_module(nc.m)
    try:
        return bass_utils.run_bass_kernel_spmd(
            nc, in_maps, core_ids=list(range(len(in_maps))),
            trace=trace, tmpdir=tmpdir)
    finally:
        nc.m = old_m


def kernel(**inputs):
    vertex = np.asarray(inputs["vertex"], dtype=np.float32)
    edge_row = np.asarray(inputs["edge_row"])
    edge_col = np.asarray(inputs["edge_col"])
    edge_val = np.asarray(inputs["edge_val"], dtype=np.float32)
    weights = np.asarray(inputs["weights"], dtype=np.float32)
    bias = np.asarray(inputs["bias"], dtype=np.float32)

    cols_t, dloc_t, vals_t, incs, nch = _preprocess(edge_row, edge_col, edge_val)
    nc = _build_nc(nch, incs)
    in_maps = _make_in_maps(vertex, weights, bias, cols_t, dloc_t, vals_t)
    res = _run(nc, in_maps)
    return np.concatenate([res.results[c]["out"] for c in range(N_CORES)], axis=0)

